# revision 57
# baseline (speedup 1.0000x reference)
"""TRN2 Bass kernel for nn_MultiHeadAttention (B=4, S=2048, D=1024, H=16, DH=64).

Sharding (8 cores): core c -> batch b = c//2, head-half hh = c%2 (8 heads each).

v3 changes vs v2 (390us -> target ~300us):
  - PV matmul FLIPPED: stationary at[:, qt*128:+128] ([k,q] chunk), moving
    vall 65 cols (64 v features + ones) -> 65 cycles per (st, qt) instead of
    1024 per st: PV drops 262K -> 133K PE cycles.  Denominator lands in psum
    col 64 per q-partition for free.
  - Softmax normalize per-partition on DVE (reciprocal + tensor_scalar mult
    with per-partition scalar) -> the ones-broadcast matmuls (16K PE cycles)
    are gone.
  - ctx comes out [q, f]; transposed into ctxT [f, q] pair-wise ([128 q,
    128 f] tiles: two heads' 64-wide ctx side by side) via DMA-xbar
    transpose (free on PE/ACT/DVE; 8 DMAs per (pair, qb)).
  - PE emission paced against ACT's exp cadence (2491 cycles/st-tile):
    scores/exp first each slot, then deadline fills (kT/qT/v/proj), then
    lazy PV drains (eligible once vall[st] exists), then out-projection
    partial/finish units pulled as padding.  Sweep1 shrunk to ~12us
    (q pair0, k pair0 sb0-1, v st0-3) so ACT starts ~22us earlier.

Per-core layout otherwise as v2: V_aug ones column per head, scores
transposed (stationary kT, moving qT), out = sum_p ctxT_chunk.T @ Wo with
pairs 0-2 partials + pair-3 finishes (+ bo/2; host sums half-cores).
All matmuls bf16 1 cycle/row.  walrus accepts ONE sync-wait per
instruction -> legalize_waits splits extras into NoOps.
"""

import sys

if "/opt/trn_rl_repo" not in sys.path:
    sys.path.insert(0, "/opt/trn_rl_repo")

import numpy as np

import concourse.bass as bass
import concourse.mybir as mybir
import concourse.tile as tile
from concourse.bass_utils import run_bass_kernel_spmd

F32 = mybir.dt.float32
BF16 = mybir.dt.bfloat16
EXP = mybir.ActivationFunctionType.Exp

B, S_FULL, D, H = 4, 2048, 1024, 16
DH = 64
NCORES = 8

ACT_CY_PER_ST = 2491  # exp [128,1024] busy at PE-cycle equivalents (1038ns)


def legalize_waits(nc, max_waits=1):
    """Split >max_waits sync-waits per instruction into single-wait NoOps on
    the same engine, placed immediately before (per-engine order preserved)."""
    n = 0
    for fn in nc.m.functions:
        for blk in fn.blocks:
            out = []
            for inst in blk.instructions:
                si = inst.sync_info
                if si is not None and len(si.on_wait) > max_waits:
                    waits = list(si.on_wait)
                    for w in waits[:-max_waits]:
                        nop = mybir.InstNoOp(
                            name=f"WSPLIT-{n}", ins=[], outs=[], engine=inst.engine
                        )
                        n += 1
                        nop.sync_info = mybir.SyncInfo(on_wait=[w], on_update=[])
                        out.append(nop)
                    inst.sync_info = mybir.SyncInfo(
                        on_wait=waits[-max_waits:], on_update=list(si.on_update)
                    )
                out.append(inst)
            blk.instructions[:] = out
    return n


def _bcast_ap(src_ap, parts=128):
    """Partition-broadcast a [1, N] AP to [parts, N] via a step-0 dim."""
    return bass.AP(
        tensor=src_ap.tensor,
        offset=src_ap.offset,
        ap=[[0, parts], list(src_ap.ap[-1])],
    )


def build_nc(S=S_FULL, legalize=True):
    NQB = S // 1024  # 1024-wide sq blocks
    NST = S // 128   # sk tiles
    NSB = S // 512   # 512-wide s blocks (projection granularity)
    nc = bass.Bass()
    xT_d = nc.dram_tensor("xt", [D, S], BF16, kind="ExternalInput")
    wq_d = nc.dram_tensor("wq", [128, 4096], BF16, kind="ExternalInput")
    wk_d = nc.dram_tensor("wk", [128, 4096], BF16, kind="ExternalInput")
    wv_d = nc.dram_tensor("wv", [128, 4096], BF16, kind="ExternalInput")
    wo_d = nc.dram_tensor("wo", [128, 4096], BF16, kind="ExternalInput")
    bqk_d = nc.dram_tensor("bqk", [128, 8], F32, kind="ExternalInput")
    bv_d = nc.dram_tensor("bv", [1, 512], F32, kind="ExternalInput")
    bo_d = nc.dram_tensor("bo", [1, 1024], F32, kind="ExternalInput")
    out_d = nc.dram_tensor("out", [S, 1024], BF16, kind="ExternalOutput")

    with tile.TileContext(nc) as tc, nc.allow_low_precision(
        reason="bf16 matmul inputs are intentional"
    ):
        with tc.tile_pool(name="persist", bufs=1) as pp, \
             tc.tile_pool(name="psS", bufs=2, space="PSUM") as psS, \
             tc.tile_pool(name="psP", bufs=2, space="PSUM") as psP, \
             tc.tile_pool(name="pvp", bufs=2, space="PSUM") as pvp, \
             tc.tile_pool(name="w2p", bufs=1) as w2p, \
             tc.tile_pool(name="aw", bufs=1) as aw, \
             tc.tile_pool(name="bc", bufs=1) as bc, \
             tc.tile_pool(name="at", bufs=30) as atp, \
             tc.tile_pool(name="sm", bufs=2) as sm, \
             tc.tile_pool(name="csb", bufs=1) as csb, \
             tc.tile_pool(name="co", bufs=1) as co, \
             tc.tile_pool(name="cot", bufs=2) as cot:
            qT = pp.tile([128, 4 * S], BF16)
            kT = pp.tile([128, 4 * S], BF16)
            vall = pp.tile([128, NST * 520], BF16)  # per s-tile: 8 heads x 65
            bqk = pp.tile([128, 8], F32)
            bv_b = pp.tile([128, 512], F32)
            bo_b = pp.tile([128, 1024], F32)
            wq2 = w2p.tile([128, 3072], BF16)
            wk2 = w2p.tile([128, 3072], BF16)
            wq0 = aw.tile([128, 1024], BF16)
            wk0 = aw.tile([128, 1024], BF16)
            wv = aw.tile([128, 4096], BF16)
            ctxT = bc.tile([128, 4 * S], BF16)
            wo = co.tile([128, 4096], BF16)
            xp_cm = tc.tile_pool(name="xp", bufs=1)
            xp = xp_cm.__enter__()
            xsb = [
                xp.tile([128, 4096], BF16, tag=f"xsb{sb}", name=f"xsb{sb}")
                for sb in range(NSB)
            ]

            def dma_w_cols(dst, src_d, col0, width):
                """One strided DMA: dst[:, ch*width + c] = src[p, ch*512+col0+c]."""
                src = bass.AP(
                    tensor=src_d,
                    offset=col0,
                    ap=[[4096, 128], [512, 8], [1, width]],
                )
                nc.sync.dma_start(out=dst, in_=src)

            def dma_x_half(sb, half):
                """Half-sb x load: chunks ch in [4*half, 4*half+4), 512 cols."""
                src = bass.AP(
                    tensor=xT_d,
                    offset=(4 * half) * 128 * S + sb * 512,
                    ap=[[S, 128], [128 * S, 4], [1, 512]],
                )
                nc.sync.dma_start(
                    out=xsb[sb][:, 4 * half * 512:(4 * half + 4) * 512], in_=src
                )

            # ---- input DMAs, first-needed first ----
            nc.scalar.dma_start(out=wq0[:, 0:128], in_=wq_d[:, 0:128])
            nc.sync.dma_start(out=xsb[0][:, 0:512], in_=xT_d[0:128, 0:512])
            nc.scalar.dma_start(
                out=wq0[:, 128:1024],
                in_=bass.AP(tensor=wq_d, offset=512, ap=[[4096, 128], [512, 7], [1, 128]]),
            )
            nc.sync.dma_start(out=xsb[0][:, 512:1024], in_=xT_d[128:256, 0:512])
            nc.scalar.dma_start(
                out=wk0,
                in_=bass.AP(tensor=wk_d, offset=0, ap=[[4096, 128], [512, 8], [1, 128]]),
            )
            nc.sync.dma_start(
                out=xsb[0][:, 1024:2048],
                in_=bass.AP(
                    tensor=xT_d, offset=2 * 128 * S, ap=[[S, 128], [128 * S, 2], [1, 512]]
                ),
            )
            nc.scalar.dma_start(out=bqk, in_=bqk_d[:, :])
            dma_x_half(0, 1)
            dma_x_half(1, 0)
            dma_x_half(1, 1)
            nc.sync.dma_start(out=wv, in_=wv_d[:, :])
            nc.sync.dma_start(out=bv_b, in_=_bcast_ap(bv_d[:, :]))
            dma_x_half(2, 0)
            dma_x_half(2, 1)
            dma_x_half(3, 0)
            dma_x_half(3, 1)
            # V_aug ones columns (65th col per head) via Pool memset
            nc.gpsimd.memset(
                vall.rearrange("p (t h e) -> p t h e", h=8, e=65)[:, :, :, 64:65],
                1.0,
            )

            # ---- PE-cycle pacing bookkeeping ----
            state = {"pe": 0, "act_sched": 0}

            def pe(cy):
                state["pe"] += cy

            def qk_group(wmat, wcol0, dstT, bcol, p, sb):
                stride = wmat.shape[1] // 8
                ps_q = psP.tile([128, 512], F32, tag="pp", name="ps_q")
                for ch in range(8):
                    nc.tensor.matmul(
                        ps_q,
                        wmat[:, wcol0 + ch * stride: wcol0 + ch * stride + 128],
                        xsb[sb][:, ch * 512:(ch + 1) * 512],
                        start=(ch == 0),
                        stop=(ch == 7),
                    )
                nc.vector.tensor_scalar_add(
                    dstT[:, p * S + sb * 512: p * S + (sb + 1) * 512],
                    ps_q,
                    bqk[:, bcol + p: bcol + p + 1],
                )
                pe(4096)

            def v_group(st):
                sb, t4 = st // 4, st % 4
                ps_v = psP.tile([128, 512], F32, tag="pp", name="ps_v")
                for ch in range(8):
                    nc.tensor.matmul(
                        ps_v,
                        xsb[sb][:, ch * 512 + t4 * 128: ch * 512 + (t4 + 1) * 128],
                        wv[:, ch * 512:(ch + 1) * 512],
                        start=(ch == 0),
                        stop=(ch == 7),
                    )
                dst = vall[:, st * 520:(st + 1) * 520].rearrange(
                    "p (h e) -> p h e", e=65
                )[:, :, 0:64]
                nc.vector.tensor_add(
                    dst,
                    ps_v.rearrange("p (h e) -> p h e", e=64),
                    bv_b.rearrange("p (h e) -> p h e", e=64),
                )
                pe(4096)
                state["last_v"] = st

            state["last_v"] = -1

            # ---- attention machinery ----
            # PSUM accumulation groups are zero-region (2KB bank) granular:
            # only ONE pending group per bank.  So PV drains are qt-MAJOR:
            # after a block's 16 exps, each qt accumulates over all 16 at
            # tiles back-to-back in a single short-lived [128,65] group,
            # normalizes, and moves on.  pvp bufs=2 double-buffers units.
            drain_q = []               # fifo of (h, qb, col0, width, lqt)
            norm_done_cols = {}        # (h, qb) -> set of normed col0 halves
            transposed = {}            # (pair, qb, half) -> emission slot
            block_state = {}           # (h, qb, col0) -> {"ats": [...]}
            pair_cs = {}               # (pair, qb) -> ctx_sb tile
            state["at_live"] = 0
            state["max_pending"] = 0

            AT_CAP = 29

            def score_exp(h, qb, st, col0=0, width=1024):
                # keep the at pool from clobbering un-drained tiles
                while state["at_live"] >= AT_CAP:
                    if not drain_one():
                        raise AssertionError(
                            f"at-pool full with undrainable PV at "
                            f"(h{h} qb{qb} st{st}), last_v={state['last_v']}"
                        )
                p, r0 = h // 2, 64 * (h % 2)
                base = p * S + qb * 1024 + col0
                ps_s = psS.tile([128, 1024], F32, tag="ps", name="ps_s")
                for off in range(0, width, 512):
                    nc.tensor.matmul(
                        ps_s[:, off:off + 512],
                        kT[r0:r0 + 64, p * S + st * 128: p * S + (st + 1) * 128],
                        qT[r0:r0 + 64, base + off: base + off + 512],
                        start=True,
                        stop=True,
                    )
                if width == 1024:
                    at = atp.tile([128, 1024], BF16, tag="at", name="at")
                    off = 0
                else:
                    # pack two 512-wide exps into one at tile so half-blocks
                    # don't flood the at pool
                    ha = state.get("half_at")
                    if ha is None:
                        at, off = atp.tile([128, 1024], BF16, tag="at",
                                           name="at"), 0
                        state["half_at"] = at
                    else:
                        at, off = ha, 512
                        state["half_at"] = None
                nc.scalar.activation(at[:, off:off + width], ps_s[:, 0:width],
                                     EXP, scale=0.125)
                pe(width)
                state["act_sched"] += 2 * width + 444
                key = (h, qb, col0)
                bs = block_state.setdefault(key, {"ats": [], "units": 0})
                bs["ats"].append((at, off))
                state["at_live"] += 1 if off == 0 else 0
                state["max_pending"] = max(state["max_pending"],
                                           state["at_live"])
                if st == NST - 1:
                    for lqt in range(width // 128):
                        drain_q.append((h, qb, col0, width, lqt))

            def drain_one():
                """Accumulate one (block, qt) PV unit over all 16 sts and
                normalize it.  Needs the full vall resident."""
                if not drain_q or state["last_v"] < NST - 1:
                    return False
                h, qb, col0, width, lqt = drain_q.pop(0)
                key = (h, qb, col0)
                bs = block_state[key]
                pvq = pvp.tile([128, 65], F32, tag="pv", name="pvq")
                for st in range(NST):
                    at, off = bs["ats"][st]
                    nc.tensor.matmul(
                        pvq,
                        at[:, off + lqt * 128: off + (lqt + 1) * 128],
                        vall[:, st * 520 + h * 65: st * 520 + (h + 1) * 65],
                        start=(st == 0),
                        stop=(st == NST - 1),
                    )
                pe(65 * NST)
                norm_qt(h, qb, col0, lqt, pvq)
                bs["units"] += 1
                if bs["units"] == width // 128:
                    state["at_live"] -= len(bs["ats"])
                    bs["ats"] = []
                return True

            def norm_qt(h, qb, col0, lqt, pvq):
                """DVE: reciprocal of denominator + per-partition scale."""
                p, r0 = h // 2, 64 * (h % 2)
                if (p, qb) not in pair_cs:
                    pair_cs[(p, qb)] = csb.tile(
                        [128, 1024], BF16, tag=f"csb{qb}",
                        name=f"csb_p{p}q{qb}",
                    )
                cs = pair_cs[(p, qb)]
                qt = col0 // 128 + lqt
                rcp = sm.tile([128, 1], F32, tag="rcp")
                nc.vector.reciprocal(rcp, pvq[:, 64:65])
                nc.vector.tensor_scalar_mul(
                    cs[:, qt * 128 + r0: qt * 128 + r0 + 64],
                    pvq[:, 0:64],
                    rcp,
                )
                mine = norm_done_cols.setdefault((h, qb), set())
                mine.add(qt)
                if qt in norm_done_cols.get((h ^ 1, qb), set()):
                    transpose_qt(p, qb, qt)

            def transpose_qt(p, qb, qt):
                """One DMA-xbar transpose: ctx_sb [128 q, 128 f] -> ctxT.
                Issued on the SP queue (ACT/DVE queue waits would stall
                exp/norm dispatch)."""
                cs = pair_cs[(p, qb)]
                nc.sync.dma_start(
                    out=ctxT[:, p * S + qb * 1024 + qt * 128:
                             p * S + qb * 1024 + (qt + 1) * 128],
                    in_=cs[:, qt * 128:(qt + 1) * 128],
                    transpose=True,
                )
                transposed[(p, qb, qt)] = state.get("slot", 0)
                if all((p, qb, q) in transposed for q in range(8)):
                    pair_cs.pop((p, qb))

            # ---- out-projection units (pulled as padding) ----
            partials = {}
            out_work = [("partial", t) for t in range(16)]
            out_work += [("finish", t) for t in range(16)]
            op_holder = {}

            TP_DELAY = 4  # slots for a transpose DMA chain to complete

            def tp_ok(p, qb, qt):
                k = (p, qb, qt)
                if k not in transposed:
                    return False
                return state.get("tail") or (
                    state.get("slot", 0) - transposed[k] >= TP_DELAY)

            def out_eligible(kind, t):
                qb, qt = t // 8, t % 8
                if kind == "partial":
                    return "op" in op_holder and all(
                        tp_ok(p, qb, qt) for p in range(3))
                return t in partials and tp_ok(3, qb, qt)

            def partial_tile(t):
                op = op_holder["op"]
                pt = op.tile([128, 1024], BF16, tag=f"pt{t}", name=f"pt{t}")
                partials[t] = pt
                for half in range(2):
                    ps_oh = psP.tile([128, 512], F32, tag="pp", name="ps_oh")
                    for p in range(3):
                        nc.tensor.matmul(
                            ps_oh,
                            ctxT[:, p * S + t * 128: p * S + (t + 1) * 128],
                            wo[:, p * 1024 + half * 512:
                               p * 1024 + (half + 1) * 512],
                            start=(p == 0),
                            stop=(p == 2),
                        )
                    nc.vector.tensor_add(
                        pt[:, half * 512:(half + 1) * 512],
                        ps_oh,
                        bo_b[:, half * 512:(half + 1) * 512],
                    )
                pe(3072)

            def finish_tile(t):
                if state.get("tail"):
                    # attention is done -> psS is free: both halves into one
                    # wide psum tile, one DVE add, one store (halves the
                    # per-tile WAR/sem serialization)
                    ps_f = psS.tile([128, 1024], F32, tag="ps", name="ps_f")
                    for half in range(2):
                        nc.tensor.matmul(
                            ps_f[:, half * 512:(half + 1) * 512],
                            ctxT[:, 3 * S + t * 128: 3 * S + (t + 1) * 128],
                            wo[:, 3 * 1024 + half * 512:
                               3 * 1024 + (half + 1) * 512],
                            start=True,
                            stop=True,
                        )
                    ot = cot.tile([128, 1024], BF16, tag="otw", name="otw")
                    nc.vector.scalar_tensor_tensor(
                        ot, ps_f, 1.0, partials[t],
                        mybir.AluOpType.mult, mybir.AluOpType.add,
                    )
                    nc.sync.dma_start(
                        out=out_d[t * 128:(t + 1) * 128, :], in_=ot,
                    )
                    pe(1024)
                    return
                for half in range(2):
                    ps_fh = psP.tile([128, 512], F32, tag="pp", name="ps_fh")
                    nc.tensor.matmul(
                        ps_fh,
                        ctxT[:, 3 * S + t * 128: 3 * S + (t + 1) * 128],
                        wo[:, 3 * 1024 + half * 512:
                           3 * 1024 + (half + 1) * 512],
                        start=True,
                        stop=True,
                    )
                    ot = cot.tile([128, 512], BF16, tag="oth", name="oth")
                    nc.vector.scalar_tensor_tensor(
                        ot, ps_fh, 1.0,
                        partials[t][:, half * 512:(half + 1) * 512],
                        mybir.AluOpType.mult, mybir.AluOpType.add,
                    )
                    nc.sync.dma_start(
                        out=out_d[t * 128:(t + 1) * 128,
                                  half * 512:(half + 1) * 512],
                        in_=ot,
                    )
                pe(1024)

            def pull_out_unit():
                for idx, (kind, t) in enumerate(out_work):
                    if out_eligible(kind, t):
                        out_work.pop(idx)
                        state.setdefault("pulls", []).append(
                            (state.get("where"), kind, t))
                        if kind == "partial":
                            partial_tile(t)
                        else:
                            finish_tile(t)
                        return True
                return False

            # ---- static fill schedule (deadline-driven) ----
            # units: ("q"/"k", pair, sb) projection groups or ("v", st)
            def q_(p, sb):
                return lambda: qk_group(
                    wq0 if p == 0 else wq2, 0 if p == 0 else (p - 1) * 128,
                    qT, 0, p, sb)

            def k_(p, sb):
                return lambda: qk_group(
                    wk0 if p == 0 else wk2, 0 if p == 0 else (p - 1) * 128,
                    kT, 4, p, sb)

            def v_(st):
                return lambda: v_group(st)

            # Deadlines (bi = 2h + qb; pair p first block = B_{4p}):
            #   q p sb0,sb1 + k p sb0 before B_{4p} st0; k sb1/2/3 before
            #   st4/8/12; q sb2,sb3 before B_{4p+1} st0.  v st must all land
            #   by ~B2 so PV drains can keep the at-pool bounded.
            static_fills = {  # bi -> [(st, fill), ...]
                0: [(1, k_(0, 1)), (2, v_(0)), (3, k_(0, 2)), (4, v_(1)),
                    (5, v_(2)), (6, k_(0, 3)), (7, v_(3)), (8, v_(4)),
                    (9, v_(5)), (10, v_(6)), (11, v_(7)), (12, q_(0, 2)),
                    (14, q_(0, 3))],
                1: [(0, v_(8)), (1, v_(9)), (2, v_(10)), (3, v_(11)),
                    (4, v_(12)), (5, v_(13)), (6, v_(14)), (7, v_(15))],
                2: [(4, q_(1, 0)), (10, q_(1, 1))],
                3: [(4, k_(1, 0)), (10, k_(1, 1))],
                4: [(4, k_(1, 2)), (7, k_(1, 3)), (10, q_(1, 2)),
                    (13, q_(1, 3))],
                6: [(5, q_(2, 0)), (10, k_(2, 0))],
                7: [(5, q_(2, 1)), (10, k_(2, 1))],
                8: [(4, k_(2, 2)), (7, k_(2, 3)), (10, q_(2, 2)),
                    (13, q_(2, 3))],
                9: [(5, q_(3, 0)), (10, k_(3, 0))],
                10: [(4, q_(3, 1)), (9, k_(3, 1)), (14, k_(3, 2))],
                11: [(4, k_(3, 3)), (9, q_(3, 2)), (14, q_(3, 3))],
            }

            def pump():
                """Per slot: up to 3 PV drain units (they free the at pool
                and unblock norms/transposes), then at most one out unit
                when PE is ahead of ACT pace (bursts starve the scores
                cadence and stall ACT)."""
                if drain_one():
                    pass
                pulled = 0
                while pulled < 2:
                    slack = state["act_sched"] - state["pe"]
                    if slack < 3072 or not pull_out_unit():
                        break
                    pulled += 1

            # ---- sweep1: minimal pre-attention work ----
            qk_group(wq0, 0, qT, 0, 0, 0)
            qk_group(wq0, 0, qT, 0, 0, 1)
            qk_group(wk0, 0, kT, 4, 0, 0)
            state["act_sched"] = state["pe"]  # ACT starts after sweep1

            # late weight loads (needed from B3 / B12 onwards)
            dma_w_cols(wq2, wq_d, 128, 384)
            dma_w_cols(wk2, wk_d, 128, 384)
            nc.sync.dma_start(out=bo_b, in_=_bcast_ap(bo_d[:, :]))
            nc.sync.dma_start(out=wo, in_=wo_d[:, :])

            # ---- main block loop ----
            # B0..B13 full-width; the last head's blocks (h7 qb0/qb1) run as
            # 512-wide half passes so their norms/transposes/finishes overlap
            # the remaining attention instead of serializing in the tail.
            block_list = [(bi // 2, bi % 2, 0, 1024) for bi in range(14)]
            block_list += [(7, 0, 0, 512), (7, 0, 512, 512),
                           (7, 1, 0, 512), (7, 1, 512, 512)]
            slot = 0
            for bi, (h, qb, col0, width) in enumerate(block_list):
                fill_at = {}
                for pos, f in static_fills.get(bi, []):
                    fill_at.setdefault(pos, []).append(f)
                for st in range(NST):
                    state["where"] = (bi, st)
                    state["slot"] = slot
                    slot += 1
                    for f in fill_at.get(st, []):
                        f()
                    score_exp(h, qb, st, col0, width)
                    pump()
                if bi == 11:
                    # last projection fill done -> free xsb, open partials
                    xp_cm.__exit__(None, None, None)
                    op_cm = tc.tile_pool(name="op", bufs=1)
                    op_holder["op"] = op_cm.__enter__()
                    op_holder["cm"] = op_cm

            # ---- tail: drain everything, close out ----
            state["where"] = ("tail", 0)
            state["slot"] = 10 ** 9
            state["tail"] = True
            while drain_q or out_work:
                progressed = drain_one()
                while pull_out_unit():
                    progressed = True
                if not progressed and out_work and not drain_q:
                    raise AssertionError(
                        f"stuck out work: {out_work[:4]}; "
                        f"tp={sorted(k for k in transposed if k[0] == 3)}; "
                        f"nd7={sorted(norm_done_cols.get((7, 1), set()))}; "
                        f"nd6={sorted(norm_done_cols.get((6, 1), set()))}; "
                        f"partials={sorted(partials)}")
            import os
            if os.environ.get("KDEBUG"):
                print("max_pending:", state["max_pending"])
                print("pulls:", state["pulls"])
            op_holder["cm"].__exit__(None, None, None)

    if legalize:
        legalize_waits(nc)
    return nc


def pack_core_inputs(c, x, Wq, bq, Wk, bk, Wv, bv, Wo, bo, S=S_FULL):
    """Pack full-model inputs into core c's device tensors."""
    import ml_dtypes
    BF = ml_dtypes.bfloat16
    b = c // 2
    hh = c % 2
    hs = slice(hh * 8, hh * 8 + 8)

    def pack_w(W):  # [8, D, DH] -> [128, 4096]: free = chunk*512 + (h*64+dh)
        W2 = np.transpose(W, (1, 0, 2)).reshape(D, 512)      # [d, h*dh]
        return np.ascontiguousarray(
            np.transpose(W2.reshape(8, 128, 512), (1, 0, 2)).reshape(128, 4096)
        )

    xT = np.ascontiguousarray(x[b].T)                         # [D, S]
    wq = pack_w(Wq[hs])
    wk = pack_w(Wk[hs])
    wv = pack_w(Wv[hs])
    # Wo rows for this half's features: [512, 1024] -> [128, 4*1024]
    Wr = Wo[hh * 512:(hh + 1) * 512]
    wo = np.ascontiguousarray(
        np.transpose(Wr.reshape(4, 128, 1024), (1, 0, 2)).reshape(128, 4096)
    )
    bqk = np.concatenate(
        [bq[hs].reshape(4, 128).T, bk[hs].reshape(4, 128).T], axis=1
    )                                                         # [128, 8]
    bvp = bv[hs].reshape(1, 512)
    bop = (0.5 * bo).reshape(1, 1024)
    return {
        "xt": xT.astype(BF),
        "wq": wq.astype(BF),
        "wk": wk.astype(BF),
        "wv": wv.astype(BF),
        "wo": wo.astype(BF),
        "bqk": np.ascontiguousarray(bqk).astype(np.float32),
        "bv": bvp.astype(np.float32),
        "bo": bop.astype(np.float32),
    }


_NC_CACHE = {}


def _get_nc(S=S_FULL):
    if S not in _NC_CACHE:
        _NC_CACHE[S] = build_nc(S)
    return _NC_CACHE[S]


def kernel(x, Wq, bq, Wk, bk, Wv, bv, Wo, bo, _trace=False):
    x, Wq, bq, Wk, bk, Wv, bv, Wo, bo = (
        np.asarray(a, dtype=np.float32) for a in (x, Wq, bq, Wk, bk, Wv, bv, Wo, bo)
    )
    nc = _get_nc()
    in_maps = [
        pack_core_inputs(c, x, Wq, bq, Wk, bk, Wv, bv, Wo, bo) for c in range(NCORES)
    ]
    res = run_bass_kernel_spmd(nc, in_maps, list(range(NCORES)), trace=_trace)
    out = np.empty((B, S_FULL, D), dtype=np.float32)
    for b in range(B):
        out[b] = res.results[2 * b]["out"].astype(np.float32) + \
            res.results[2 * b + 1]["out"].astype(np.float32)
    if _trace:
        kernel.last_results = res
    return out


# revision 61
# speedup vs baseline: 1.0394x; 1.0394x over previous
"""TRN2 Bass kernel for nn_MultiHeadAttention (B=4, S=2048, D=1024, H=16, DH=64).

Sharding (8 cores): core c -> batch b = c//2, head-half hh = c%2 (8 heads each).

v3 changes vs v2 (390244 -> 371000 ns graded):
  - PV matmul FLIPPED: stationary at[:, qt*128:+128] ([k,q] chunk), moving
    vall 65 cols (64 v features + ones) -> 65 cycles per (st, qt) instead of
    1024 per st: PV drops 262K -> 133K PE cycles.  Denominator lands in psum
    col 64 per q-partition for free.
  - PSUM accumulation groups are zero-region (2KB bank) granular -> PV
    drains are qt-MAJOR: after a block's 16 exps, each qt accumulates over
    all 16 at tiles in one short-lived [128,65] group (pvp bufs=2), paced
    one unit per st slot through the next block (at pool 30 bufs).
  - Softmax normalize per-partition on DVE (reciprocal + tensor_scalar mult
    with per-partition scalar) -> the ones-broadcast matmuls are gone.
  - ctx comes out [q, f]; transposed into ctxT [f, q] pair-wise ([128 q,
    128 f] blocks: two heads' 64-wide ctx side by side) via DMA-xbar
    transpose on the SP queue, one DMA per qt as both heads' norms land.
  - PE emission paced against ACT's exp cadence: scores/exp first each
    slot, then deadline fills (kT/qT/v/proj at explicit (block, st)
    positions), one PV drain unit, then out-projection partial/finish
    units (per-qt eligibility, TP_DELAY slots after the transpose DMA so
    PE never head-of-line-waits the xbar chain).  h7's two blocks run as
    512-wide half passes (two half-exps packed per at tile) so their
    norms/transposes/finishes overlap attention instead of serializing in
    the tail; tail finishes use wide [128,1024] psS tiles, one DVE add +
    one store each.

Per-core layout otherwise as v2: V_aug ones column per head, scores
transposed (stationary kT, moving qT), out = sum_p ctxT_chunk.T @ Wo with
pairs 0-2 partials + pair-3 finishes (+ bo/2; host sums half-cores).
All matmuls bf16 1 cycle/row.  walrus accepts ONE sync-wait per
instruction -> legalize_waits splits extras into NoOps.
"""

import sys

if "/opt/trn_rl_repo" not in sys.path:
    sys.path.insert(0, "/opt/trn_rl_repo")

import numpy as np

import concourse.bass as bass
import concourse.mybir as mybir
import concourse.tile as tile
from concourse.bass_utils import run_bass_kernel_spmd

F32 = mybir.dt.float32
BF16 = mybir.dt.bfloat16
EXP = mybir.ActivationFunctionType.Exp

B, S_FULL, D, H = 4, 2048, 1024, 16
DH = 64
NCORES = 8

ACT_CY_PER_ST = 2491  # exp [128,1024] busy at PE-cycle equivalents (1038ns)


def legalize_waits(nc, max_waits=1):
    """Split >max_waits sync-waits per instruction into single-wait NoOps on
    the same engine, placed immediately before (per-engine order preserved)."""
    n = 0
    for fn in nc.m.functions:
        for blk in fn.blocks:
            out = []
            for inst in blk.instructions:
                si = inst.sync_info
                if si is not None and len(si.on_wait) > max_waits:
                    waits = list(si.on_wait)
                    for w in waits[:-max_waits]:
                        nop = mybir.InstNoOp(
                            name=f"WSPLIT-{n}", ins=[], outs=[], engine=inst.engine
                        )
                        n += 1
                        nop.sync_info = mybir.SyncInfo(on_wait=[w], on_update=[])
                        out.append(nop)
                    inst.sync_info = mybir.SyncInfo(
                        on_wait=waits[-max_waits:], on_update=list(si.on_update)
                    )
                out.append(inst)
            blk.instructions[:] = out
    return n


def _bcast_ap(src_ap, parts=128):
    """Partition-broadcast a [1, N] AP to [parts, N] via a step-0 dim."""
    return bass.AP(
        tensor=src_ap.tensor,
        offset=src_ap.offset,
        ap=[[0, parts], list(src_ap.ap[-1])],
    )


def build_nc(S=S_FULL, legalize=True):
    NQB = S // 1024  # 1024-wide sq blocks
    NST = S // 128   # sk tiles
    NSB = S // 512   # 512-wide s blocks (projection granularity)
    nc = bass.Bass()
    xT_d = nc.dram_tensor("xt", [D, S], BF16, kind="ExternalInput")
    wq_d = nc.dram_tensor("wq", [128, 4096], BF16, kind="ExternalInput")
    wk_d = nc.dram_tensor("wk", [128, 4096], BF16, kind="ExternalInput")
    wv_d = nc.dram_tensor("wv", [128, 4096], BF16, kind="ExternalInput")
    wo_d = nc.dram_tensor("wo", [128, 4096], BF16, kind="ExternalInput")
    bqk_d = nc.dram_tensor("bqk", [128, 8], F32, kind="ExternalInput")
    bv_d = nc.dram_tensor("bv", [1, 512], F32, kind="ExternalInput")
    bo_d = nc.dram_tensor("bo", [1, 1024], F32, kind="ExternalInput")
    out_d = nc.dram_tensor("out", [S, 1024], BF16, kind="ExternalOutput")

    with tile.TileContext(nc) as tc, nc.allow_low_precision(
        reason="bf16 matmul inputs are intentional"
    ):
        with tc.tile_pool(name="persist", bufs=1) as pp, \
             tc.tile_pool(name="psS", bufs=2, space="PSUM") as psS, \
             tc.tile_pool(name="psP", bufs=2, space="PSUM") as psP, \
             tc.tile_pool(name="pvp", bufs=2, space="PSUM") as pvp, \
             tc.tile_pool(name="w2p", bufs=1) as w2p, \
             tc.tile_pool(name="aw", bufs=1) as aw, \
             tc.tile_pool(name="bc", bufs=1) as bc, \
             tc.tile_pool(name="at", bufs=30) as atp, \
             tc.tile_pool(name="sm", bufs=2) as sm, \
             tc.tile_pool(name="csb", bufs=1) as csb, \
             tc.tile_pool(name="co", bufs=1) as co, \
             tc.tile_pool(name="cot", bufs=2) as cot:
            qT = pp.tile([128, 4 * S], BF16)
            kT = pp.tile([128, 4 * S], BF16)
            vall = pp.tile([128, NST * 520], BF16)  # per s-tile: 8 heads x 65
            bqk = pp.tile([128, 8], F32)
            bv_b = pp.tile([128, 512], F32)
            bo_b = pp.tile([128, 1024], F32)
            wq2 = w2p.tile([128, 3072], BF16)
            wk2 = w2p.tile([128, 3072], BF16)
            wq0 = aw.tile([128, 1024], BF16)
            wk0 = aw.tile([128, 1024], BF16)
            wv = aw.tile([128, 4096], BF16)
            ctxT = bc.tile([128, 4 * S], BF16)
            wo = co.tile([128, 4096], BF16)
            xp_cm = tc.tile_pool(name="xp", bufs=1)
            xp = xp_cm.__enter__()
            xsb = [
                xp.tile([128, 4096], BF16, tag=f"xsb{sb}", name=f"xsb{sb}")
                for sb in range(NSB)
            ]

            def dma_w_cols(dst, src_d, col0, width):
                """One strided DMA: dst[:, ch*width + c] = src[p, ch*512+col0+c]."""
                src = bass.AP(
                    tensor=src_d,
                    offset=col0,
                    ap=[[4096, 128], [512, 8], [1, width]],
                )
                nc.sync.dma_start(out=dst, in_=src)

            def dma_x_half(sb, half):
                """Half-sb x load: chunks ch in [4*half, 4*half+4), 512 cols."""
                src = bass.AP(
                    tensor=xT_d,
                    offset=(4 * half) * 128 * S + sb * 512,
                    ap=[[S, 128], [128 * S, 4], [1, 512]],
                )
                nc.sync.dma_start(
                    out=xsb[sb][:, 4 * half * 512:(4 * half + 4) * 512], in_=src
                )

            # ---- input DMAs, first-needed first ----
            nc.scalar.dma_start(out=wq0[:, 0:128], in_=wq_d[:, 0:128])
            nc.sync.dma_start(out=xsb[0][:, 0:512], in_=xT_d[0:128, 0:512])
            nc.scalar.dma_start(
                out=wq0[:, 128:1024],
                in_=bass.AP(tensor=wq_d, offset=512, ap=[[4096, 128], [512, 7], [1, 128]]),
            )
            nc.sync.dma_start(out=xsb[0][:, 512:1024], in_=xT_d[128:256, 0:512])
            nc.scalar.dma_start(
                out=wk0,
                in_=bass.AP(tensor=wk_d, offset=0, ap=[[4096, 128], [512, 8], [1, 128]]),
            )
            nc.sync.dma_start(
                out=xsb[0][:, 1024:2048],
                in_=bass.AP(
                    tensor=xT_d, offset=2 * 128 * S, ap=[[S, 128], [128 * S, 2], [1, 512]]
                ),
            )
            nc.scalar.dma_start(out=bqk, in_=bqk_d[:, :])
            dma_x_half(0, 1)
            dma_x_half(1, 0)
            dma_x_half(1, 1)
            nc.sync.dma_start(out=wv, in_=wv_d[:, :])
            nc.sync.dma_start(out=bv_b, in_=_bcast_ap(bv_d[:, :]))
            dma_x_half(2, 0)
            dma_x_half(2, 1)
            dma_x_half(3, 0)
            dma_x_half(3, 1)
            # V_aug ones columns (65th col per head) via Pool memset
            nc.gpsimd.memset(
                vall.rearrange("p (t h e) -> p t h e", h=8, e=65)[:, :, :, 64:65],
                1.0,
            )

            # ---- PE-cycle pacing bookkeeping ----
            state = {"pe": 0, "act_sched": 0}

            def pe(cy):
                state["pe"] += cy

            def qk_group(wmat, wcol0, dstT, bcol, p, sb):
                stride = wmat.shape[1] // 8
                ps_q = psP.tile([128, 512], F32, tag="pp", name="ps_q")
                for ch in range(8):
                    nc.tensor.matmul(
                        ps_q,
                        wmat[:, wcol0 + ch * stride: wcol0 + ch * stride + 128],
                        xsb[sb][:, ch * 512:(ch + 1) * 512],
                        start=(ch == 0),
                        stop=(ch == 7),
                    )
                nc.vector.tensor_scalar_add(
                    dstT[:, p * S + sb * 512: p * S + (sb + 1) * 512],
                    ps_q,
                    bqk[:, bcol + p: bcol + p + 1],
                )
                pe(4096)

            def v_group(st, p):
                """v projection for one head-PAIR at one s-tile: a PV drain
                unit for head h only reads vall cols h*65..h*65+65, so pair
                p's v is only needed just before pair p's first drain."""
                sb, t4 = st // 4, st % 4
                ps_vf = psP.tile([128, 512], F32, tag="pp", name="ps_vf")
                ps_v = ps_vf[:, 0:128]
                for ch in range(8):
                    nc.tensor.matmul(
                        ps_v,
                        xsb[sb][:, ch * 512 + t4 * 128: ch * 512 + (t4 + 1) * 128],
                        wv[:, ch * 512 + p * 128: ch * 512 + (p + 1) * 128],
                        start=(ch == 0),
                        stop=(ch == 7),
                    )
                dst = vall[:, st * 520 + p * 130: st * 520 + (p + 1) * 130]
                dst = dst.rearrange("p (h e) -> p h e", e=65)[:, :, 0:64]
                nc.vector.tensor_add(
                    dst,
                    ps_v.rearrange("p (h e) -> p h e", e=64),
                    bv_b[:, p * 128:(p + 1) * 128].rearrange(
                        "p (h e) -> p h e", e=64),
                )
                pe(1024)
                state["last_vp"][p] += 1

            state["last_vp"] = [0, 0, 0, 0]

            # ---- attention machinery ----
            # PSUM accumulation groups are zero-region (2KB bank) granular:
            # only ONE pending group per bank.  So PV drains are qt-MAJOR:
            # after a block's 16 exps, each qt accumulates over all 16 at
            # tiles back-to-back in a single short-lived [128,65] group,
            # normalizes, and moves on.  pvp bufs=2 double-buffers units.
            drain_q = []               # fifo of (h, qb, col0, width, lqt)
            norm_done_cols = {}        # (h, qb) -> set of normed col0 halves
            transposed = {}            # (pair, qb, half) -> emission slot
            block_state = {}           # (h, qb, col0) -> {"ats": [...]}
            pair_cs = {}               # (pair, qb) -> ctx_sb tile
            state["at_live"] = 0
            state["max_pending"] = 0

            AT_CAP = 29

            def score_exp(h, qb, st, col0=0, width=1024):
                # keep the at pool from clobbering un-drained tiles
                while state["at_live"] >= AT_CAP:
                    if not drain_one():
                        raise AssertionError(
                            f"at-pool full with undrainable PV at "
                            f"(h{h} qb{qb} st{st}), "
                            f"last_vp={state['last_vp']}"
                        )
                p, r0 = h // 2, 64 * (h % 2)
                base = p * S + qb * 1024 + col0
                ps_s = psS.tile([128, 1024], F32, tag="ps", name="ps_s")
                for off in range(0, width, 512):
                    nc.tensor.matmul(
                        ps_s[:, off:off + 512],
                        kT[r0:r0 + 64, p * S + st * 128: p * S + (st + 1) * 128],
                        qT[r0:r0 + 64, base + off: base + off + 512],
                        start=True,
                        stop=True,
                    )
                if width == 1024:
                    at = atp.tile([128, 1024], BF16, tag="at", name="at")
                    off = 0
                else:
                    # pack two 512-wide exps into one at tile so half-blocks
                    # don't flood the at pool
                    ha = state.get("half_at")
                    if ha is None:
                        at, off = atp.tile([128, 1024], BF16, tag="at",
                                           name="at"), 0
                        state["half_at"] = at
                    else:
                        at, off = ha, 512
                        state["half_at"] = None
                nc.scalar.activation(at[:, off:off + width], ps_s[:, 0:width],
                                     EXP, scale=0.125)
                pe(width)
                state["act_sched"] += 2 * width + 444
                key = (h, qb, col0)
                bs = block_state.setdefault(key, {"ats": [], "units": 0})
                bs["ats"].append((at, off))
                state["at_live"] += 1 if off == 0 else 0
                state["max_pending"] = max(state["max_pending"],
                                           state["at_live"])
                if st == NST - 1:
                    for lqt in range(width // 128):
                        drain_q.append((h, qb, col0, width, lqt))

            def drain_one():
                """Accumulate one (block, qt) PV unit over all 16 sts and
                normalize it.  Needs the full vall resident."""
                if not drain_q:
                    return False
                h = drain_q[0][0]
                if state["last_vp"][h // 2] < NST:
                    return False
                h, qb, col0, width, lqt = drain_q.pop(0)
                key = (h, qb, col0)
                bs = block_state[key]
                pvq = pvp.tile([128, 65], F32, tag="pv", name="pvq")
                for st in range(NST):
                    at, off = bs["ats"][st]
                    nc.tensor.matmul(
                        pvq,
                        at[:, off + lqt * 128: off + (lqt + 1) * 128],
                        vall[:, st * 520 + h * 65: st * 520 + (h + 1) * 65],
                        start=(st == 0),
                        stop=(st == NST - 1),
                    )
                pe(65 * NST)
                norm_qt(h, qb, col0, lqt, pvq)
                bs["units"] += 1
                if bs["units"] == width // 128:
                    state["at_live"] -= len(bs["ats"])
                    bs["ats"] = []
                return True

            def norm_qt(h, qb, col0, lqt, pvq):
                """DVE: reciprocal of denominator + per-partition scale."""
                p, r0 = h // 2, 64 * (h % 2)
                if (p, qb) not in pair_cs:
                    pair_cs[(p, qb)] = csb.tile(
                        [128, 1024], BF16, tag=f"csb{qb}",
                        name=f"csb_p{p}q{qb}",
                    )
                cs = pair_cs[(p, qb)]
                qt = col0 // 128 + lqt
                rcp = sm.tile([128, 1], F32, tag="rcp")
                nc.vector.reciprocal(rcp, pvq[:, 64:65])
                nc.vector.tensor_scalar_mul(
                    cs[:, qt * 128 + r0: qt * 128 + r0 + 64],
                    pvq[:, 0:64],
                    rcp,
                )
                mine = norm_done_cols.setdefault((h, qb), set())
                mine.add(qt)
                if qt in norm_done_cols.get((h ^ 1, qb), set()):
                    transpose_qt(p, qb, qt)

            def transpose_qt(p, qb, qt):
                """One DMA-xbar transpose: ctx_sb [128 q, 128 f] -> ctxT.
                Issued on the SP queue (ACT/DVE queue waits would stall
                exp/norm dispatch)."""
                cs = pair_cs[(p, qb)]
                nc.sync.dma_start(
                    out=ctxT[:, p * S + qb * 1024 + qt * 128:
                             p * S + qb * 1024 + (qt + 1) * 128],
                    in_=cs[:, qt * 128:(qt + 1) * 128],
                    transpose=True,
                )
                transposed[(p, qb, qt)] = state.get("slot", 0)
                if all((p, qb, q) in transposed for q in range(8)):
                    pair_cs.pop((p, qb))

            # ---- out-projection units (pulled as padding) ----
            partials = {}
            out_work = [("partial", t) for t in range(16)]
            out_work += [("finish", t) for t in range(16)]
            op_holder = {}

            TP_DELAY = 4  # slots for a transpose DMA chain to complete

            def tp_ok(p, qb, qt):
                k = (p, qb, qt)
                if k not in transposed:
                    return False
                return state.get("tail") or (
                    state.get("slot", 0) - transposed[k] >= TP_DELAY)

            def out_eligible(kind, t):
                qb, qt = t // 8, t % 8
                if kind == "partial":
                    return "op" in op_holder and all(
                        tp_ok(p, qb, qt) for p in range(3))
                return t in partials and tp_ok(3, qb, qt)

            def partial_tile(t):
                op = op_holder["op"]
                pt = op.tile([128, 1024], BF16, tag=f"pt{t}", name=f"pt{t}")
                partials[t] = pt
                for half in range(2):
                    ps_oh = psP.tile([128, 512], F32, tag="pp", name="ps_oh")
                    for p in range(3):
                        nc.tensor.matmul(
                            ps_oh,
                            ctxT[:, p * S + t * 128: p * S + (t + 1) * 128],
                            wo[:, p * 1024 + half * 512:
                               p * 1024 + (half + 1) * 512],
                            start=(p == 0),
                            stop=(p == 2),
                        )
                    nc.vector.tensor_add(
                        pt[:, half * 512:(half + 1) * 512],
                        ps_oh,
                        bo_b[:, half * 512:(half + 1) * 512],
                    )
                pe(3072)

            def finish_tile(t):
                if state.get("tail"):
                    # attention is done -> psS is free: both halves into one
                    # wide psum tile, one DVE add, one store (halves the
                    # per-tile WAR/sem serialization)
                    ps_f = psS.tile([128, 1024], F32, tag="ps", name="ps_f")
                    for half in range(2):
                        nc.tensor.matmul(
                            ps_f[:, half * 512:(half + 1) * 512],
                            ctxT[:, 3 * S + t * 128: 3 * S + (t + 1) * 128],
                            wo[:, 3 * 1024 + half * 512:
                               3 * 1024 + (half + 1) * 512],
                            start=True,
                            stop=True,
                        )
                    ot = cot.tile([128, 1024], BF16, tag="otw", name="otw")
                    nc.vector.scalar_tensor_tensor(
                        ot, ps_f, 1.0, partials[t],
                        mybir.AluOpType.mult, mybir.AluOpType.add,
                    )
                    nc.sync.dma_start(
                        out=out_d[t * 128:(t + 1) * 128, :], in_=ot,
                    )
                    pe(1024)
                    return
                for half in range(2):
                    ps_fh = psP.tile([128, 512], F32, tag="pp", name="ps_fh")
                    nc.tensor.matmul(
                        ps_fh,
                        ctxT[:, 3 * S + t * 128: 3 * S + (t + 1) * 128],
                        wo[:, 3 * 1024 + half * 512:
                           3 * 1024 + (half + 1) * 512],
                        start=True,
                        stop=True,
                    )
                    ot = cot.tile([128, 512], BF16, tag="oth", name="oth")
                    nc.vector.scalar_tensor_tensor(
                        ot, ps_fh, 1.0,
                        partials[t][:, half * 512:(half + 1) * 512],
                        mybir.AluOpType.mult, mybir.AluOpType.add,
                    )
                    nc.sync.dma_start(
                        out=out_d[t * 128:(t + 1) * 128,
                                  half * 512:(half + 1) * 512],
                        in_=ot,
                    )
                pe(1024)

            def pull_out_unit():
                for idx, (kind, t) in enumerate(out_work):
                    if out_eligible(kind, t):
                        out_work.pop(idx)
                        state.setdefault("pulls", []).append(
                            (state.get("where"), kind, t))
                        if kind == "partial":
                            partial_tile(t)
                        else:
                            finish_tile(t)
                        return True
                return False

            # ---- static fill schedule (deadline-driven) ----
            # units: ("q"/"k", pair, sb) projection groups or ("v", st)
            def q_(p, sb):
                return lambda: qk_group(
                    wq0 if p == 0 else wq2, 0 if p == 0 else (p - 1) * 128,
                    qT, 0, p, sb)

            def k_(p, sb):
                return lambda: qk_group(
                    wk0 if p == 0 else wk2, 0 if p == 0 else (p - 1) * 128,
                    kT, 4, p, sb)

            def v_(st, p):
                return lambda: v_group(st, p)

            # Deadlines (bi = 2h + qb; pair p first block = B_{4p}):
            #   q p sb0,sb1 + k p sb0 before B_{4p} st0; k sb1/2/3 before
            #   st4/8/12; q sb2,sb3 before B_{4p+1} st0.  v st must all land
            #   by ~B2 so PV drains can keep the at-pool bounded.
            static_fills = {  # bi -> [(st, fill), ...]
                0: [(st, v_(st, 0)) for st in range(16)] + [
                    (1, k_(0, 1)), (5, k_(0, 2)), (9, k_(0, 3)),
                    (11, q_(0, 2)), (13, q_(0, 3))],
                2: [(st, v_(st, 1)) for st in range(8)] + [(9, q_(1, 0))],
                3: [(2 * i, v_(8 + i, 1)) for i in range(8)] + [
                    (9, q_(1, 1)), (11, k_(1, 0)), (13, k_(1, 1))],
                4: [(4, k_(1, 2)), (8, k_(1, 3)), (11, q_(1, 2)),
                    (13, q_(1, 3))],
                6: [(st, v_(st, 2)) for st in range(8)] + [(9, q_(2, 0))],
                7: [(2 * i, v_(8 + i, 2)) for i in range(8)] + [
                    (9, q_(2, 1)), (11, k_(2, 0)), (13, k_(2, 1))],
                8: [(4, k_(2, 2)), (8, k_(2, 3)), (11, q_(2, 2)),
                    (13, q_(2, 3))],
                10: [(st, v_(st, 3)) for st in range(8)] + [(9, q_(3, 0))],
                11: [(2 * i, v_(8 + i, 3)) for i in range(8)] + [
                    (9, q_(3, 1)), (11, k_(3, 0)), (13, k_(3, 1))],
                12: [(4, k_(3, 2)), (8, k_(3, 3)), (11, q_(3, 2)),
                    (13, q_(3, 3))],
            }

            def pump():
                """Per slot: up to 3 PV drain units (they free the at pool
                and unblock norms/transposes), then at most one out unit
                when PE is ahead of ACT pace (bursts starve the scores
                cadence and stall ACT)."""
                if drain_one():
                    pass
                pulled = 0
                while pulled < 2:
                    slack = state["act_sched"] - state["pe"]
                    if slack < 3072 or not pull_out_unit():
                        break
                    pulled += 1

            # ---- sweep1: minimal pre-attention work ----
            qk_group(wq0, 0, qT, 0, 0, 0)
            qk_group(wq0, 0, qT, 0, 0, 1)
            qk_group(wk0, 0, kT, 4, 0, 0)
            state["act_sched"] = state["pe"]  # ACT starts after sweep1

            # late weight loads (needed from B3 / B12 onwards)
            dma_w_cols(wq2, wq_d, 128, 384)
            dma_w_cols(wk2, wk_d, 128, 384)
            nc.sync.dma_start(out=bo_b, in_=_bcast_ap(bo_d[:, :]))
            nc.sync.dma_start(out=wo, in_=wo_d[:, :])

            # ---- main block loop ----
            # B0..B13 full-width; the last head's blocks (h7 qb0/qb1) run as
            # 512-wide half passes so their norms/transposes/finishes overlap
            # the remaining attention instead of serializing in the tail.
            block_list = [(bi // 2, bi % 2, 0, 1024) for bi in range(14)]
            block_list += [(7, 0, 0, 512), (7, 0, 512, 512),
                           (7, 1, 0, 512), (7, 1, 512, 512)]
            slot = 0
            for bi, (h, qb, col0, width) in enumerate(block_list):
                fill_at = {}
                for pos, f in static_fills.get(bi, []):
                    fill_at.setdefault(pos, []).append(f)
                for st in range(NST):
                    state["where"] = (bi, st)
                    state["slot"] = slot
                    slot += 1
                    for f in fill_at.get(st, []):
                        f()
                    score_exp(h, qb, st, col0, width)
                    pump()
                if bi == 12:
                    # last projection fill done -> free xsb, open partials
                    xp_cm.__exit__(None, None, None)
                    op_cm = tc.tile_pool(name="op", bufs=1)
                    op_holder["op"] = op_cm.__enter__()
                    op_holder["cm"] = op_cm

            # ---- tail: drain everything, close out ----
            state["where"] = ("tail", 0)
            state["slot"] = 10 ** 9
            state["tail"] = True
            while drain_q or out_work:
                progressed = drain_one()
                while pull_out_unit():
                    progressed = True
                if not progressed and out_work and not drain_q:
                    raise AssertionError(
                        f"stuck out work: {out_work[:4]}; "
                        f"tp={sorted(k for k in transposed if k[0] == 3)}; "
                        f"nd7={sorted(norm_done_cols.get((7, 1), set()))}; "
                        f"nd6={sorted(norm_done_cols.get((6, 1), set()))}; "
                        f"partials={sorted(partials)}")
            import os
            if os.environ.get("KDEBUG"):
                print("max_pending:", state["max_pending"])
                print("pulls:", state["pulls"])
            op_holder["cm"].__exit__(None, None, None)

    if legalize:
        legalize_waits(nc)
    return nc


def pack_core_inputs(c, x, Wq, bq, Wk, bk, Wv, bv, Wo, bo, S=S_FULL):
    """Pack full-model inputs into core c's device tensors."""
    import ml_dtypes
    BF = ml_dtypes.bfloat16
    b = c // 2
    hh = c % 2
    hs = slice(hh * 8, hh * 8 + 8)

    def pack_w(W):  # [8, D, DH] -> [128, 4096]: free = chunk*512 + (h*64+dh)
        W2 = np.transpose(W, (1, 0, 2)).reshape(D, 512)      # [d, h*dh]
        return np.ascontiguousarray(
            np.transpose(W2.reshape(8, 128, 512), (1, 0, 2)).reshape(128, 4096)
        )

    xT = np.ascontiguousarray(x[b].T)                         # [D, S]
    wq = pack_w(Wq[hs])
    wk = pack_w(Wk[hs])
    wv = pack_w(Wv[hs])
    # Wo rows for this half's features: [512, 1024] -> [128, 4*1024]
    Wr = Wo[hh * 512:(hh + 1) * 512]
    wo = np.ascontiguousarray(
        np.transpose(Wr.reshape(4, 128, 1024), (1, 0, 2)).reshape(128, 4096)
    )
    bqk = np.concatenate(
        [bq[hs].reshape(4, 128).T, bk[hs].reshape(4, 128).T], axis=1
    )                                                         # [128, 8]
    bvp = bv[hs].reshape(1, 512)
    bop = (0.5 * bo).reshape(1, 1024)
    return {
        "xt": xT.astype(BF),
        "wq": wq.astype(BF),
        "wk": wk.astype(BF),
        "wv": wv.astype(BF),
        "wo": wo.astype(BF),
        "bqk": np.ascontiguousarray(bqk).astype(np.float32),
        "bv": bvp.astype(np.float32),
        "bo": bop.astype(np.float32),
    }


_NC_CACHE = {}


def _get_nc(S=S_FULL):
    if S not in _NC_CACHE:
        _NC_CACHE[S] = build_nc(S)
    return _NC_CACHE[S]


def kernel(x, Wq, bq, Wk, bk, Wv, bv, Wo, bo, _trace=False):
    x, Wq, bq, Wk, bk, Wv, bv, Wo, bo = (
        np.asarray(a, dtype=np.float32) for a in (x, Wq, bq, Wk, bk, Wv, bv, Wo, bo)
    )
    nc = _get_nc()
    in_maps = [
        pack_core_inputs(c, x, Wq, bq, Wk, bk, Wv, bv, Wo, bo) for c in range(NCORES)
    ]
    res = run_bass_kernel_spmd(nc, in_maps, list(range(NCORES)), trace=_trace)
    out = np.empty((B, S_FULL, D), dtype=np.float32)
    for b in range(B):
        out[b] = res.results[2 * b]["out"].astype(np.float32) + \
            res.results[2 * b + 1]["out"].astype(np.float32)
    if _trace:
        kernel.last_results = res
    return out
